# revision 18
# baseline (speedup 1.0000x reference)
"""Trainium2 Bass kernel for nn_EulerFullAttention.

Math (per batch b, head h, dh=64):
  theta_q = x/(1+|w_q|) + b_q + t*phi_q ; Q = [cos(theta_q), sin(theta_q)]  (S,128)
  theta_k likewise ; K = [cos, sin]
  V = cos(theta_v)+sin(theta_v) = sqrt(2)*sin(theta_v + pi/4)              (S,64)
  scores = Q @ K^T / sqrt(128), causal softmax, out = attn @ V
  result = cos(theta_o)+sin(theta_o) = sqrt(2)*sin(theta_o + pi/4),
    theta_o = out/(1+|w_out|) + b_out

Distribution: 8 cores = 2 batches x 4 head-groups (4 heads each). Each core
computes its x[:, 256-col] slice end to end; no collectives.

Trig via range reduction: r = theta/(2*pi) + c ; f = r - round(r) in
[-0.5, 0.5] (int32 cast rounds-to-nearest) ; sin(theta) = Sin(2*pi*f).
cos adds +0.25 to c; the +pi/4 folds +0.125 into c.

Attention in transposed layout: scoresT[k, q] = KT.T @ QT with QT/KT
feature-major [128, S] (rows 0:64 cos / 64:128 sin, built by PE transpose
of x plus a partition-shifted SBUF DMA dup). exp via ACT from PSUM;
causal handled by only computing blocks with k_block <= q range, a last
affine_select zeroing the triangular boundary. attn@V accumulates
outT[65, 512] per 512-wide q chunk with lhsT = [V | 1] so row 64 gives the
softmax denominator for free. PE transposes outT back to natural layout,
normalization multiplies by 1/rowsum (and sqrt(2)).
"""

import sys, math

sys.path.insert(0, "/opt/trn_rl_repo")

import numpy as np
import concourse.bass as bass
import concourse.mybir as mybir
from concourse.bacc import Bacc
from concourse.tile import TileContext
from concourse.bass_utils import run_bass_kernel_spmd
from contextlib import ExitStack

F32 = mybir.dt.float32
I32 = mybir.dt.int32
AF = mybir.ActivationFunctionType
ALU = mybir.AluOpType

B, S, D, H = 2, 2048, 1024, 16
DH = 64
NH = 4            # heads per core
DC = NH * DH      # 256 feature columns per core
NB = S // 128     # 16 s-blocks
TWO_PI = 2.0 * math.pi
SQRT2 = math.sqrt(2.0)
EXP_SCALE = 1.0 / math.sqrt(2.0 * DH)
F32R = mybir.dt.float32r  # attention matmuls: fp32r = 1 cyc/row vs fp32's 4


def _bcast_mid(ap2d, n):
    """[128, F] AP -> [128, n, F] with stride-0 middle dim."""
    return bass.AP(tensor=ap2d.tensor, offset=ap2d.offset,
                   ap=[ap2d.ap[0], [0, n], ap2d.ap[-1]])


def _build_packs(qc):
    """PSUM pack layout for one 512-wide q chunk: list of packs, each a list
    of (kb, qs, N, off) strips placed in a [128,1024] (2-bank) psum tile."""
    order = list(range(4 * qc)) + [4 * qc, 4 * qc + 1, 4 * qc + 3, 4 * qc + 2]
    packs, cur, off = [], [], 0
    for kb in order:
        if kb < 4 * qc:
            qs, N = 512 * qc, 512
        else:
            jj = kb - 4 * qc
            qs, N = 512 * qc + 128 * jj, 512 - 128 * jj
        o = off
        if o % 512 + N > 512:
            o = (o // 512 + 1) * 512
        if o + N > 1024:
            packs.append(cur)
            cur, o = [], 0
        cur.append((kb, qs, N, o))
        off = o + N
    if cur:
        packs.append(cur)
    return packs


def build_nc(tphi_sig=(0, 0, 0, 0, 0, 0, 0, 0)):
    """tphi_sig[j*2+pi] = group id of the (iota*phi2+c2) table for head j,
    proj pi; equal ids share one on-chip table (host dedups)."""
    nc = Bacc(trn_type="TRN2")
    xin = nc.dram_tensor("xin", [S, DC], F32, kind="ExternalInput")
    qkp_d = nc.dram_tensor("qkp", [128, NH, 6], F32, kind="ExternalInput")
    vp_d = nc.dram_tensor("vp", [128, 2, DC], F32, kind="ExternalInput")
    op_d = nc.dram_tensor("opar", [128, 2, DC], F32, kind="ExternalInput")
    out_d = nc.dram_tensor("out", [S, DC], F32, kind="ExternalOutput")
    ident_d = nc.inline_tensor(np.eye(128, dtype=np.float32), "ident")
    iota_d = nc.inline_tensor(
        np.tile(np.arange(S, dtype=np.float32), (128, 1)), "iota")

    with TileContext(nc) as tc, ExitStack() as ctx:
        sing = ctx.enter_context(tc.tile_pool(name="sing", bufs=1))
        qkpool = ctx.enter_context(tc.tile_pool(name="qkp", bufs=5 * NH))
        mid = ctx.enter_context(tc.tile_pool(name="mid", bufs=6))
        midi = ctx.enter_context(tc.tile_pool(name="midi", bufs=2))
        otpool = ctx.enter_context(tc.tile_pool(name="otp", bufs=2))
        expool = ctx.enter_context(tc.tile_pool(name="exp", bufs=4))
        tiny = ctx.enter_context(tc.tile_pool(name="tiny", bufs=4))
        tphip = ctx.enter_context(tc.tile_pool(name="tphip", bufs=len(set(tphi_sig))))
        psp = ctx.enter_context(tc.tile_pool(name="psp", bufs=4, space="PSUM"))

        x_s = sing.tile([128, NB, DC], F32)
        nc.sync.dma_start(out=x_s, in_=xin[:, :].rearrange("(n p) d -> p n d", p=128))
        ident = sing.tile([128, 128], F32)
        nc.sync.dma_start(out=ident, in_=ident_d[:, :])
        iota = sing.tile([128, S], F32)
        nc.sync.dma_start(out=iota, in_=iota_d[:, :])
        qkp = sing.tile([128, NH, 6], F32)
        nc.sync.dma_start(out=qkp, in_=qkp_d[:, :, :])
        vp = sing.tile([128, 2, DC], F32)
        nc.sync.dma_start(out=vp, in_=vp_d[:, :, :])
        opr = sing.tile([128, 2, DC], F32)
        nc.sync.dma_start(out=opr, in_=op_d[:, :, :])
        bz = sing.tile([128, 1], F32)
        nc.vector.memset(bz, 0.0)
        onat = sing.tile([128, NB, DC], F32)
        vaug = []
        for j in range(NH):
            t = sing.tile([128, NB, DH + 1], F32R, tag=f"vaug{j}")
            nc.vector.memset(t[:, :, DH:DH + 1].bitcast(F32), 1.0)
            vaug.append(t)

        tphi_tiles = {}

        def get_tphi(j, pi):
            g = tphi_sig[2 * j + pi]
            if g not in tphi_tiles:
                c0 = 3 * pi
                tph = tphip.tile([128, S], F32, tag="tphi")
                nc.vector.tensor_scalar(out=tph, in0=iota,
                                        scalar1=qkp[:, j, c0 + 1:c0 + 2],
                                        scalar2=qkp[:, j, c0 + 2:c0 + 3],
                                        op0=ALU.mult, op1=ALU.add)
                tphi_tiles[g] = tph
            return tphi_tiles[g]

        QT, KT = [None] * NH, [None] * NH
        last_sin = [None]

        def qk_prep(j):
            x2t = mid.tile([128, S], F32, tag="mid")
            for cc in range(2):
                xtp = psp.tile([64, 1024], F32, tag="ps")
                for sb in range(8):
                    n = 8 * cc + sb
                    nc.tensor.transpose(xtp[:, 128 * sb:128 * sb + 128],
                                        x_s[:, n, DH * j:DH * j + DH], ident)
                nc.vector.tensor_copy(out=x2t[0:64, 1024 * cc:1024 * cc + 1024], in_=xtp)
                nc.sync.dma_start(out=x2t[64:128, 1024 * cc:1024 * cc + 1024],
                                  in_=x2t[0:64, 1024 * cc:1024 * cc + 1024])
            for pi in range(2):
                c0 = 3 * pi
                tph = get_tphi(j, pi)
                r2 = mid.tile([128, S], F32, tag="mid")
                nc.vector.scalar_tensor_tensor(out=r2, in0=x2t,
                                               scalar=qkp[:, j, c0:c0 + 1], in1=tph,
                                               op0=ALU.mult, op1=ALU.add)
                i2 = midi.tile([128, S], I32, tag="midi")
                nc.gpsimd.tensor_copy(out=i2, in_=r2)
                f2 = mid.tile([128, S], F32, tag="mid")
                nc.vector.scalar_tensor_tensor(out=f2, in0=i2, scalar=-1.0, in1=r2,
                                               op0=ALU.mult, op1=ALU.add)
                t = qkpool.tile([128, S], F32R, tag="qk")
                last_sin[0] = nc.scalar.activation(out=t, in_=f2, func=AF.Sin,
                                                   bias=bz[:, 0:1], scale=TWO_PI)
                if pi == 0:
                    QT[j] = t
                else:
                    KT[j] = t

        def v_phase():
            for hh in range(2):
                xh = x_s[:, 8 * hh:8 * hh + 8, :]
                rv = mid.tile([128, 8, DC], F32, tag="mid")
                nc.gpsimd.tensor_tensor(out=rv, in0=xh, in1=_bcast_mid(vp[:, 0, :], 8), op=ALU.mult)
                nc.gpsimd.tensor_tensor(out=rv, in0=rv, in1=_bcast_mid(vp[:, 1, :], 8), op=ALU.add)
                iv = midi.tile([128, 8, DC], I32, tag="midi")
                nc.gpsimd.tensor_copy(out=iv, in_=rv)
                nc.vector.scalar_tensor_tensor(out=rv, in0=iv, scalar=-1.0, in1=rv,
                                               op0=ALU.mult, op1=ALU.add)
                sv = mid.tile([128, 8, DC], F32, tag="mid")
                nc.scalar.activation(out=sv, in_=rv, func=AF.Sin, bias=bz[:, 0:1], scale=TWO_PI)
                for j in range(NH):
                    nc.vector.tensor_copy(out=vaug[j][:, 8 * hh:8 * hh + 8, 0:DH],
                                          in_=sv[:, :, DH * j:DH * j + DH])

        def attention(j, order_dep):
            first_exp = None
            for qc in range(4):
                ot_ps = psp.tile([65, 512], F32, tag="ps")
                packs = _build_packs(qc)
                n_av = 4 * qc + 4
                avi = 0
                for pack in packs:
                    sc = psp.tile([128, 1024], F32, tag="ps")
                    for (kb, qs, N, off) in pack:
                        nc.tensor.matmul(sc[:, off:off + N],
                                         KT[j][:, 128 * kb:128 * kb + 128],
                                         QT[j][:, qs:qs + N],
                                         start=True, stop=True)
                    width = pack[-1][3] + pack[-1][2]
                    ext = expool.tile([128, 1024], F32R, tag="ex")
                    e = nc.scalar.activation(out=ext[:, 0:width], in_=sc[:, 0:width],
                                             func=AF.Exp, bias=bz[:, 0:1], scale=EXP_SCALE)
                    if first_exp is None and order_dep is not None:
                        bass._add_dep_helper(e.ins, order_dep.ins, sync=True,
                                             reason="act-table-order")
                    if first_exp is None:
                        first_exp = e
                    for (kb, qs, N, off) in pack:
                        if kb >= 4 * qc:  # diagonal strip: zero exp where q < k
                            nc.gpsimd.affine_select(
                                out=ext[:, off:off + 128], in_=ext[:, off:off + 128],
                                pattern=[[1, 128]], compare_op=ALU.is_ge, fill=0.0,
                                base=0, channel_multiplier=-1)
                    for (kb, qs, N, off) in pack:
                        q0 = qs - 512 * qc
                        nc.tensor.matmul(ot_ps[:, q0:q0 + N],
                                         vaug[j][:, kb, :],
                                         ext[:, off:off + N],
                                         start=(avi == 0), stop=(avi == n_av - 1))
                        avi += 1
                ot_s = otpool.tile([65, 512], F32, tag="ot")
                nc.vector.tensor_copy(out=ot_s, in_=ot_ps)
                on_ps = psp.tile([128, 4, DH + 1], F32, tag="ps")
                for t4 in range(4):
                    nc.tensor.transpose(on_ps[:, t4, :], ot_s[:, 128 * t4:128 * t4 + 128],
                                        ident[0:65, 0:65])
                rec = tiny.tile([128, 4], F32, tag="tiny")
                nc.vector.reciprocal(out=rec, in_=on_ps[:, :, DH:DH + 1])
                for t4 in range(4):
                    nc.vector.tensor_scalar(
                        out=onat[:, 4 * qc + t4, DH * j:DH * j + DH],
                        in0=on_ps[:, t4, 0:DH], scalar1=rec[:, t4:t4 + 1],
                        scalar2=SQRT2, op0=ALU.mult, op1=ALU.mult)

        # order: prep(0); V; prep(1); att(0); att(1); prep(2); prep(3); att(2); att(3)
        qk_prep(0)
        v_phase()
        qk_prep(1)
        dep01 = last_sin[0]
        attention(0, dep01)
        attention(1, None)
        qk_prep(2)
        qk_prep(3)
        dep23 = last_sin[0]
        attention(2, dep23)
        attention(3, None)

        # ---------------- final layer (2 halves) ----------------
        out_r = out_d[:, :].rearrange("(n p) d -> p n d", p=128)
        for hh in range(2):
            ro = mid.tile([128, 8, DC], F32, tag="mid")
            nc.vector.tensor_tensor(out=ro, in0=onat[:, 8 * hh:8 * hh + 8, :],
                                    in1=_bcast_mid(opr[:, 0, :], 8), op=ALU.mult)
            nc.vector.tensor_tensor(out=ro, in0=ro, in1=_bcast_mid(opr[:, 1, :], 8), op=ALU.add)
            io = midi.tile([128, 8, DC], I32, tag="midi")
            nc.gpsimd.tensor_copy(out=io, in_=ro)
            nc.vector.scalar_tensor_tensor(out=ro, in0=io, scalar=-1.0, in1=ro,
                                           op0=ALU.mult, op1=ALU.add)
            nc.scalar.activation(out=ro, in_=ro, func=AF.Sin, bias=bz[:, 0:1], scale=TWO_PI)
            nc.vector.tensor_scalar(out=ro, in0=ro, scalar1=SQRT2, scalar2=None, op0=ALU.mult)
            nc.sync.dma_start(out=out_r[:, 8 * hh:8 * hh + 8, :], in_=ro)

    nc.finalize()
    return nc


def _host_params(inputs, c):
    """Per-core input dict for core c."""
    b, g = c // 4, c % 4
    inv2pi = 1.0 / (2.0 * np.pi)
    x = np.asarray(inputs["x"], dtype=np.float32)
    xin = np.ascontiguousarray(x[b, :, DC * g:DC * g + DC])

    def f64(a):
        return np.asarray(a, dtype=np.float64)

    qkp = np.zeros((128, NH, 6), dtype=np.float32)
    rows = np.arange(128) % DH
    cos_row = (np.arange(128) < DH).astype(np.float64) * 0.25
    for j in range(NH):
        h = NH * g + j
        for pi, (wn, bn, pn) in enumerate([("w_q", "b_q", "phi_q"),
                                           ("w_k", "b_k", "phi_k")]):
            w = f64(inputs[wn])[h]
            bb = f64(inputs[bn])[h]
            ph = f64(inputs[pn])[h]
            qkp[:, j, 3 * pi + 0] = (inv2pi / (1.0 + np.abs(w)))[rows]
            qkp[:, j, 3 * pi + 1] = (ph * inv2pi)[rows]
            qkp[:, j, 3 * pi + 2] = (bb * inv2pi)[rows] + cos_row

    vp = np.zeros((128, 2, DC), dtype=np.float32)
    wv = f64(inputs["w_v"])[NH * g:NH * g + NH].reshape(-1)
    bv = f64(inputs["b_v"])[NH * g:NH * g + NH].reshape(-1)
    vp[:, 0, :] = (inv2pi / (1.0 + np.abs(wv)))[None, :]
    vp[:, 1, :] = (bv * inv2pi + 0.125)[None, :]

    op = np.zeros((128, 2, DC), dtype=np.float32)
    wo = f64(inputs["w_out"])[DC * g:DC * g + DC]
    bo = f64(inputs["b_out"])[DC * g:DC * g + DC]
    op[:, 0, :] = (inv2pi / (1.0 + np.abs(wo)))[None, :]
    op[:, 1, :] = (bo * inv2pi + 0.125)[None, :]

    return {"xin": xin, "qkp": qkp, "vp": vp, "opar": op}


def _add_tphi(m, sig):
    # tphi[g][p, s] = f32(s*phi2[p] + c2[p]) for each group rep, in f64
    ngroups = len(set(sig))
    if ngroups > 2:
        return m
    qkp = np.asarray(m["qkp"], dtype=np.float64)
    tphi = np.zeros((ngroups, 128, S), dtype=np.float32)
    done = set()
    s_arr = np.arange(S, dtype=np.float64)
    for j in range(NH):
        for pi in range(2):
            g = sig[2 * j + pi]
            if g in done:
                continue
            done.add(g)
            c0 = 3 * pi
            phi2 = qkp[:, j, c0 + 1]
            c2 = qkp[:, j, c0 + 2]
            tphi[g] = (s_arr[None, :] * phi2[:, None] + c2[:, None]).astype(np.float32)
    m = dict(m)
    m["tphi"] = tphi
    return m


_NC_CACHE = {}


def _tphi_signature(qkp):
    cols = []
    for j in range(NH):
        for pi in range(2):
            cols.append(qkp[:, j, (3 * pi + 1, 3 * pi + 2)].tobytes())
    uniq = {}
    return tuple(uniq.setdefault(c, len(uniq)) for c in cols)


def kernel(**inputs) -> np.ndarray:
    in_maps = [_host_params(inputs, c) for c in range(8)]
    sigs = {_tphi_signature(m["qkp"]) for m in in_maps}
    sig = sigs.pop() if len(sigs) == 1 else tuple(range(2 * NH))
    in_maps = [_add_tphi(m, sig) for m in in_maps]
    if _NC_CACHE.get("sig") != sig:
        _NC_CACHE["nc"] = build_nc(sig)
        _NC_CACHE["sig"] = sig
    nc = _NC_CACHE["nc"]
    res = run_bass_kernel_spmd(nc, in_maps, core_ids=list(range(8)))
    full = np.empty((B, S, D), dtype=np.float32)
    for c in range(8):
        b, g = c // 4, c % 4
        full[b, :, DC * g:DC * g + DC] = res.results[c]["out"]
    return full
